# revision 15
# baseline (speedup 1.0000x reference)
"""Trainium2 Bass kernel for nn_AttentionDecoder (N=100000, H=256, 8 cores).

Math reduction (same as the fp16 baseline)
------------------------------------------
With W_ks = W_static_kvl[:, :H] etc., the reference collapses to one pass
over the only large tensors (h_static, h_dynamic):

    compat   = h_s @ u_s + h_d @ u_d        with u_* = (W_k* @ q)/sqrt(H)
    p_i      = exp(compat_i - SHIFT)        (valid nodes only)
    context  = ((t_s) @ W_vs + (t_d) @ W_vd) / s,  t = sum_i p_i [h_s|h_d]_i

The host compacts to the ~50% valid rows before sharding.  Pad rows are
zero; exp(0-SHIFT)=e^-8=3.35e-4 rounds to 0 in the fp8 p-grid, so pads
contribute exactly nothing to t or s (no host-side pad correction).

v27 (fp8): the node stream is float8e4 (1 B/elem), halving HBM traffic vs
fp16.  Per 128x512 tile, compat runs on one of two paths:
  * DVE: one fused scalar_tensor_tensor (mult + row-reduce) vs a broadcast
    fp8 u row.  fp8 costs the same DVE cycles as fp16 (no 2x mode either
    way), so the byte halving is free.
  * PE: the host also ships those tiles TRANSPOSED (col-major chunks); each
    chunk is an FWL weight load + a 1-column matmul against u-chunk,
    accumulating compat for 128 rows in PSUM.  Sustained cost is ~25ns per
    chunk, so PE absorbs most tiles; the extra bytes (tile shipped twice)
    still total well under the fp16 single-copy budget.
Each block gets two batched exps (SBUF cblk for DVE tiles, PSUM cp for PE
tiles) writing p = exp(compat-SHIFT) straight into a 16B-strided fp8 p-grid
(the stride satisfies DoubleRow's lhsT step%16 rule).  Weighted sums then
run as fp8 DoubleRow matmuls: one matmul per PAIR of tiles (lhsT = two
p-columns, rhs = two adjacent tiles), ~2x the fp16 rate, rotating 3 PSUM
row-groups; s = sum(p) via a ones-column matmul over the strided p-grid.
DMA: hh blocks stream on the Sync HWDGE queue, the transposed packs +
singles on the GpSimd queue (otherwise idle).  Host runs the tiny MLP head
and exact jax sampling, as before.
"""

import math

import numpy as np
import ml_dtypes

import concourse.bacc as bacc
import concourse.mybir as mybir
import concourse.tile as tile
from concourse import bass_utils

# ---- problem constants (hardcoded per harness contract) ----
H = 256
NCORES = 8
P = 128                     # SBUF partitions
BMAX = 8                    # max tiles per DMA block
SHIFT = 8.0
NEG = np.float32(-1e9)
FP8 = ml_dtypes.float8_e4m3

# test.py hooks
TRACE_OPTS: dict = {}
LAST_RESULTS = None
LAST_INTERNALS: dict = {}

_prog_cache: dict = {}


def _make_plan(tiles):
    """Static schedule for a per-core tile count.

    Blocks have even sizes (DoubleRow pairs tiles within a block); a final
    odd tile gets its own 1-tile block.  Within each block the first nv
    tiles take the DVE compat path, the rest the PE (transposed) path.
    Returns dict with sizes, per-block (nv, np), p_tiles (global indices of
    PE-path tiles, in pack order).
    """
    assert tiles % 2 == 0 or tiles == 1
    # ramp: small first block (all DVE: no transposed pack needed yet so
    # compute starts on the first hh bytes), fat middle, tiny tail blocks
    sizes = []
    rem = tiles
    for r in (4, 6):
        if rem <= 0:
            break
        s = min(r, rem)
        sizes.append(s)
        rem -= s
    while rem > 4:
        s = min(BMAX, rem - 4)
        sizes.append(s)
        rem -= s
    while rem > 0:
        s = min(2, rem)
        sizes.append(s)
        rem -= s

    nblk = len(sizes)
    # global DVE-path share (engine balance; see module docstring)
    n_dve = int(round(0.40 * tiles))
    if tiles < 6:
        n_dve = tiles

    nv = [0] * nblk
    # first block entirely DVE, then spread the rest evenly
    nv[0] = min(sizes[0], n_dve)
    k = nv[0]
    while k < n_dve:
        done = True
        for b in range(1, nblk):
            if k >= n_dve:
                break
            if nv[b] < sizes[b] - 1:
                nv[b] += 1
                k += 1
                done = False
        if done:
            break
    n_dve = k

    # prefer DVE on the first block, PE on the last (short PE tail)
    if nblk >= 2 and nv[-1] > 0 and sizes[0] - nv[0] >= nv[-1]:
        nv[0] += nv[-1]
        nv[-1] = 0

    paths = []
    p_tiles = []
    t0 = 0
    for b in range(nblk):
        npb = sizes[b] - nv[b]
        paths.append((nv[b], npb))
        for j in range(npb):
            p_tiles.append(t0 + nv[b] + j)
        t0 += sizes[b]

    # ws pair -> PSUM bank; bank 2 closes early so its output copy overlaps
    # the tail blocks
    npairs = tiles // 2
    pair_bank = [0] * npairs
    for c in range(npairs):
        pair_bank[c] = c % 3 if c < npairs - 3 else (c - (npairs - 3)) % 2
    bank_last = {}
    bank_first = {}
    for c, bk in enumerate(pair_bank):
        bank_last[bk] = c
        bank_first.setdefault(bk, c)
    return dict(sizes=sizes, paths=paths, p_tiles=p_tiles, npt=len(p_tiles),
                pair_bank=pair_bank, bank_last=bank_last,
                bank_first=bank_first)


def _build_program(tiles):
    key = ("v27", tiles)
    if key in _prog_cache:
        return _prog_cache[key]

    plan = _make_plan(tiles)
    sizes, paths = plan["sizes"], plan["paths"]
    npt = plan["npt"]

    f32 = mybir.dt.float32
    f16 = mybir.dt.float16
    f8 = mybir.dt.float8e4
    nc = bacc.Bacc(
        "TRN2",
        target_bir_lowering=False,
        debug=False,
        enable_asserts=False,
        num_devices=NCORES,
        enable_partition_id=False,
        monotonic_sem_count=0,
    )
    # one DRAM tensor of 512B-per-partition units: each block's segment is
    # [its hh tiles | its transposed packs], so ONE DMA per block delivers
    # both, alternating between the Sync and Scalar HWDGE queues
    units = tiles + npt
    blk = nc.dram_tensor("blk", [P, units, 2 * H], f8,
                         kind="ExternalInput").ap()
    # u broadcast row (512) | u chunk-major (4) | ones (1) | pad
    ubx = nc.dram_tensor("ubx", [P, 2 * H + 8], f8, kind="ExternalInput").ap()
    t_out = nc.dram_tensor("t_out", [1, 6 * H + 1], f32,
                           kind="ExternalOutput").ap()

    npairs = tiles // 2
    pair_bank = plan["pair_bank"]
    bank_last = plan["bank_last"]
    bank_first = plan["bank_first"]

    with tile.TileContext(nc) as tc:
        with (
            tc.tile_pool(name="singles", bufs=1) as singles,
            tc.tile_pool(name="blocks", bufs=5) as blocks,
            tc.tile_pool(name="small", bufs=2) as small,
            tc.tile_pool(name="scratch", bufs=4) as scratch,
            tc.tile_pool(name="psum", bufs=1, space="PSUM") as psum,
            tc.tile_pool(name="psc", bufs=2, space="PSUM") as psc,
        ):
            ubx_sb = singles.tile([P, 2 * H + 8], f8)
            u_sb = ubx_sb[:, 0:2 * H]
            ones_sb = ubx_sb[:, 2 * H + 4:2 * H + 5]
            p_grid = singles.tile([P, tiles, 16], f8)
            nshift = singles.tile([P, 1], f32)
            nc.gpsimd.memset(nshift, -SHIFT)

            t_banks = [psum.tile([1, 2 * H], f32, tag=f"tall{i}",
                                 name=f"tall{i}") for i in range(3)]
            s_ps = psum.tile([1, tiles], f32, tag="sps")

            t_sb = small.tile([1, 6 * H + 1], f32, tag="tsb")
            banks_copied = set()

            def emit_ws(pt0, psz, pbuf):
                for g in range(0, psz - 1, 2):
                    c = (pt0 + g) // 2
                    bk = pair_bank[c]
                    nc.tensor.matmul(
                        t_banks[bk],
                        lhsT=p_grid[:, pt0 + g:pt0 + g + 2, 0:1],
                        rhs=pbuf[:, g:g + 2, :],
                        start=(c == bank_first[bk]),
                        stop=(c == bank_last[bk]),
                        perf_mode=mybir.MatmulPerfMode.DoubleRow,
                    )
                    if c == bank_last[bk]:
                        # bank closed: drain it now so the copy overlaps
                        # the remaining stream
                        nc.vector.tensor_copy(
                            t_sb[0:1, 2 * H * bk:2 * H * (bk + 1)],
                            t_banks[bk])
                        banks_copied.add(bk)

            pending = []
            nblk = len(sizes)
            kP = 0   # global PE-tile serial (pack order)
            t0 = 0
            for b, sz in enumerate(sizes):
                nV, nP = paths[b]
                if b == 0:
                    nc.scalar.dma_start(out=ubx_sb, in_=ubx)
                bt = blocks.tile([P, 2 * BMAX, 2 * H], f8)
                eng = nc.sync if b % 2 == 0 else nc.scalar
                eng.dma_start(out=bt[:, 0:sz + nP, :],
                              in_=blk[:, t0 + kP:t0 + kP + sz + nP, :])

                # deferred weighted sums first: they are ready (exp done
                # blocks ago) so PE never idles waiting for this block's data
                defer = 1 if b <= 2 else 2
                while len(pending) > defer:
                    emit_ws(*pending.pop(0))

                # DVE path: fused multiply + row-reduce per tile
                cblk = scratch.tile([P, BMAX], f32, tag="cblk")
                for g in range(nV):
                    sc = scratch.tile([P, 2 * H], f16, tag="sttout")
                    nc.vector.scalar_tensor_tensor(
                        out=sc,
                        in0=bt[:, g, :],
                        scalar=1.0,
                        in1=u_sb,
                        op0=mybir.AluOpType.mult,
                        op1=mybir.AluOpType.mult,
                        accum_out=cblk[:, g:g + 1],
                    )
                # PE path: per tile, 4 FWL weight loads + 1-col matmuls
                if nP > 0:
                    cp = psc.tile([P, BMAX], f32, tag="cp")
                    for j in range(nP):
                        for ch in range(4):
                            nc.tensor.matmul(
                                cp[:, j:j + 1],
                                lhsT=bt[:, sz + j, ch * P:(ch + 1) * P],
                                rhs=ubx_sb[:, 2 * H + ch:2 * H + ch + 1],
                                start=(ch == 0),
                                stop=(ch == 3),
                            )
                # batched exps -> fp8 p-grid (16B stride)
                if nV > 0:
                    nc.scalar.activation(
                        out=p_grid[:, t0:t0 + nV, 0],
                        in_=cblk[:, 0:nV],
                        func=mybir.ActivationFunctionType.Exp,
                        bias=nshift,
                        scale=1.0,
                    )
                if nP > 0:
                    nc.scalar.activation(
                        out=p_grid[:, t0 + nV:t0 + sz, 0],
                        in_=cp[:, 0:nP],
                        func=mybir.ActivationFunctionType.Exp,
                        bias=nshift,
                        scale=1.0,
                    )
                kP += nP
                pending.append((t0, sz, bt))
                t0 += sz

            while pending:
                emit_ws(*pending.pop(0))

            # s = sum(p): partition-reduce via ones-matmul over the strided
            # p-grid, then a tiny free-dim reduce on the [1, tiles] PSUM row
            nc.tensor.matmul(s_ps, lhsT=ones_sb, rhs=p_grid[:, 0:tiles, 0],
                             start=True, stop=True)
            for bk in range(3):
                if bk not in banks_copied:
                    nc.vector.tensor_copy(
                        t_sb[0:1, 2 * H * bk:2 * H * (bk + 1)], t_banks[bk])
            nc.vector.reduce_sum(out=t_sb[0:1, 6 * H:], in_=s_ps,
                                 axis=mybir.AxisListType.X)
            nc.sync.dma_start(out=t_out, in_=t_sb)

    nc.compile()
    _prog_cache[key] = (nc, plan)
    return nc, plan


def _run_device(h_static, h_dynamic, u_cat, valid_idx):
    """Stream the compacted valid rows through the 8-core SPMD kernel.

    Returns (t [2H] float64 summed over cores, s float64).  Pad rows
    contribute exactly zero (their fp8 p rounds to 0), so no correction.
    """
    global LAST_RESULTS

    nv = len(valid_idx)
    q = (nv + NCORES - 1) // NCORES
    tiles = max(1, (q + P - 1) // P)
    if tiles % 2:
        tiles += 1          # even count: DoubleRow pairs tiles, no odd path
    npad = P * tiles
    nc, plan = _build_program(tiles)
    p_tiles = plan["p_tiles"]
    npt = plan["npt"]

    u8 = u_cat.astype(FP8)
    ubx = np.zeros((P, 2 * H + 8), FP8)
    ubx[:, 0:2 * H] = u8
    ubx[:, 2 * H:2 * H + 4] = u8.reshape(4, P).T
    ubx[:, 2 * H + 4] = FP8(1.0)

    sizes, paths = plan["sizes"], plan["paths"]
    units = tiles + npt
    in_maps = []
    for c in range(NCORES):
        rows = valid_idx[c * q:(c + 1) * q]
        nr = len(rows)
        h8 = np.zeros((npad, 2 * H), FP8)
        if nr:
            h8[:nr, 0:H] = h_static[rows].astype(FP8)
            h8[:nr, H:2 * H] = h_dynamic[rows].astype(FP8)
        hview = h8.reshape(P, tiles, 2 * H)
        blk = np.zeros((P, units, 2 * H), FP8)
        t0 = kp = 0
        for b, sz in enumerate(sizes):
            nV, nP = paths[b]
            seg = t0 + kp
            blk[:, seg:seg + sz, :] = hview[:, t0:t0 + sz, :]
            for j in range(nP):
                tr = hview[:, t0 + nV + j, :]      # [P, 2H] rows of tile
                for ch in range(4):
                    blk[:, seg + sz + j, ch * P:(ch + 1) * P] = \
                        tr[:, ch * P:(ch + 1) * P].T
            t0 += sz
            kp += nP
        in_maps.append({"blk": blk, "ubx": ubx})

    res = bass_utils.run_bass_kernel_spmd(
        nc, in_maps, core_ids=list(range(NCORES)), **TRACE_OPTS
    )
    LAST_RESULTS = res

    t = np.zeros(2 * H, np.float64)
    s = 0.0
    for c in range(NCORES):
        arr = res.results[c]["t_out"].astype(np.float64)[0]
        t += arr[0:2 * H] + arr[2 * H:4 * H] + arr[4 * H:6 * H]
        s += arr[6 * H]
    return t, s


def kernel(
    h_dynamic,
    h_static,
    W_static_kvl,
    W_dyn_kvl,
    W_q,
    W1,
    b1,
    W2,
    b2,
    valid_mask,
    current_node,
):
    h_dynamic = np.asarray(h_dynamic, np.float32)
    h_static = np.asarray(h_static, np.float32)
    W_static_kvl = np.asarray(W_static_kvl, np.float32)
    W_dyn_kvl = np.asarray(W_dyn_kvl, np.float32)
    W_q = np.asarray(W_q, np.float32)
    W1 = np.asarray(W1, np.float32)
    b1 = np.asarray(b1, np.float32)
    W2 = np.asarray(W2, np.float32)
    b2 = np.asarray(b2, np.float32)
    valid = np.asarray(valid_mask).astype(bool)
    cur = int(current_node)

    scale = 1.0 / math.sqrt(H)

    # ---- tiny host-side prologue (exact math on one row) ----
    h_cur = (h_static[cur].astype(np.float64) + h_dynamic[cur].astype(np.float64))
    q = h_cur @ W_q.astype(np.float64)  # [H]
    u_s = (W_static_kvl[:, 0:H].astype(np.float64) @ q) * scale
    u_d = (W_dyn_kvl[:, 0:H].astype(np.float64) @ q) * scale
    u_cat = np.concatenate([u_s, u_d]).astype(np.float32)  # [2H]

    valid_idx = np.flatnonzero(valid)

    W_vs = W_static_kvl[:, H:2 * H].astype(np.float64)
    W_vd = W_dyn_kvl[:, H:2 * H].astype(np.float64)

    if len(valid_idx) == 0:
        # all-masked edge case: reference softmax degenerates to uniform
        # over all N nodes; context is the mean of V. The logit cancels in
        # the final output anyway; run the device on a dummy row for timing.
        t, s = _run_device(h_static, h_dynamic, u_cat, np.array([0]))
        context = (h_static.mean(0).astype(np.float64) @ W_vs
                   + h_dynamic.mean(0).astype(np.float64) @ W_vd)
    else:
        t, s = _run_device(h_static, h_dynamic, u_cat, valid_idx)
        context = (t[:H] @ W_vs + t[H:] @ W_vd) / s  # [H]

    # ---- tiny host-side epilogue ----
    fuse = np.concatenate([h_cur, context])  # [2H]
    hidden = np.maximum(fuse @ W1.astype(np.float64) + b1.astype(np.float64), 0.0)
    logit = float(hidden @ W2.astype(np.float64)[:, 0] + float(b2[0]))

    logits_all = np.where(valid, np.float32(logit), NEG).astype(np.float32)

    LAST_INTERNALS.update(
        dict(u_cat=u_cat, t=t, s=s, context=context, logit=logit)
    )

    # exact replication of the reference's sampling (jax threefry, key(1))
    import contextlib

    import jax
    import jax.numpy as jnp

    try:
        ctx = jax.default_device(jax.devices("cpu")[0])
    except Exception:
        ctx = contextlib.nullcontext()
    with ctx:
        logits_j = jnp.asarray(logits_all)
        choice = jax.random.categorical(jax.random.key(1), logits_j)
        log_probs = jax.nn.log_softmax(logits_j)
        log_prob = log_probs[choice]
        choice_np = np.asarray(choice)
        log_prob_np = np.asarray(log_prob)

    return (choice_np, log_prob_np)


# revision 16
# speedup vs baseline: 1.1328x; 1.1328x over previous
"""Trainium2 Bass kernel for nn_AttentionDecoder (N=100000, H=256, 8 cores).

Math reduction (same as the fp16 baseline)
------------------------------------------
With W_ks = W_static_kvl[:, :H] etc., the reference collapses to one pass
over the only large tensors (h_static, h_dynamic):

    compat   = h_s @ u_s + h_d @ u_d        with u_* = (W_k* @ q)/sqrt(H)
    p_i      = exp(compat_i - SHIFT)        (valid nodes only)
    context  = ((t_s) @ W_vs + (t_d) @ W_vd) / s,  t = sum_i p_i [h_s|h_d]_i

The host compacts to the ~50% valid rows before sharding.  Pad rows are
zero; exp(0-SHIFT)=e^-8=3.35e-4 rounds to 0 in the fp8 p-grid, so pads
contribute exactly nothing to t or s (no host-side pad correction).

v27 (fp8): the node stream is float8e4 (1 B/elem), halving HBM traffic vs
fp16.  Per 128x512 tile, compat runs on one of two paths:
  * DVE: one fused scalar_tensor_tensor (mult + row-reduce) vs a broadcast
    fp8 u row.  fp8 costs the same DVE cycles as fp16 (no 2x mode either
    way), so the byte halving is free.
  * PE: the host also ships those tiles TRANSPOSED (col-major chunks); each
    chunk is an FWL weight load + a 1-column matmul against u-chunk,
    accumulating compat for 128 rows in PSUM.  Sustained cost is ~25ns per
    chunk, so PE absorbs most tiles; the extra bytes (tile shipped twice)
    still total well under the fp16 single-copy budget.
Each block gets two batched exps (SBUF cblk for DVE tiles, PSUM cp for PE
tiles) writing p = exp(compat-SHIFT) straight into a 16B-strided fp8 p-grid
(the stride satisfies DoubleRow's lhsT step%16 rule).  Weighted sums then
run as fp8 DoubleRow matmuls: one matmul per PAIR of tiles (lhsT = two
p-columns, rhs = two adjacent tiles), ~2x the fp16 rate, rotating 3 PSUM
row-groups; s = sum(p) via a ones-column matmul over the strided p-grid.
DMA: hh blocks stream on the Sync HWDGE queue, the transposed packs +
singles on the GpSimd queue (otherwise idle).  Host runs the tiny MLP head
and exact jax sampling, as before.
"""

import math

import numpy as np
import ml_dtypes

import concourse.bacc as bacc
import concourse.mybir as mybir
import concourse.tile as tile
from concourse import bass_utils

# ---- problem constants (hardcoded per harness contract) ----
H = 256
NCORES = 8
P = 128                     # SBUF partitions
BMAX = 8                    # max tiles per DMA block
SHIFT = 8.0
NEG = np.float32(-1e9)
FP8 = ml_dtypes.float8_e4m3

# test.py hooks
TRACE_OPTS: dict = {}
LAST_RESULTS = None
LAST_INTERNALS: dict = {}

_prog_cache: dict = {}


def _make_plan(tiles):
    """Static schedule for a per-core tile count.

    Blocks have even sizes (DoubleRow pairs tiles within a block); a final
    odd tile gets its own 1-tile block.  Within each block the first nv
    tiles take the DVE compat path, the rest the PE (transposed) path.
    Returns dict with sizes, per-block (nv, np), p_tiles (global indices of
    PE-path tiles, in pack order).
    """
    assert tiles % 2 == 0 or tiles == 1
    # ramp: small first block (all DVE: no transposed pack needed yet so
    # compute starts on the first hh bytes), fat middle, tiny tail blocks
    sizes = []
    rem = tiles
    for r in (4, 6):
        if rem <= 0:
            break
        s = min(r, rem)
        sizes.append(s)
        rem -= s
    while rem > 4:
        s = min(BMAX, rem - 4)
        sizes.append(s)
        rem -= s
    while rem > 0:
        s = min(2, rem)
        sizes.append(s)
        rem -= s

    nblk = len(sizes)
    # global DVE-path share (engine balance; see module docstring)
    n_dve = int(round(0.40 * tiles))
    if tiles < 6:
        n_dve = tiles

    nv = [0] * nblk
    # first block entirely DVE, then spread the rest evenly
    nv[0] = min(sizes[0], n_dve)
    k = nv[0]
    while k < n_dve:
        done = True
        for b in range(1, nblk):
            if k >= n_dve:
                break
            if nv[b] < sizes[b] - 1:
                nv[b] += 1
                k += 1
                done = False
        if done:
            break
    n_dve = k

    # prefer DVE on the first block, PE on the last (short PE tail)
    if nblk >= 2 and nv[-1] > 0 and sizes[0] - nv[0] >= nv[-1]:
        nv[0] += nv[-1]
        nv[-1] = 0

    paths = []
    p_tiles = []
    t0 = 0
    for b in range(nblk):
        npb = sizes[b] - nv[b]
        paths.append((nv[b], npb))
        for j in range(npb):
            p_tiles.append(t0 + nv[b] + j)
        t0 += sizes[b]

    # ws pair -> PSUM bank; bank 2 closes early so its output copy overlaps
    # the tail blocks
    npairs = tiles // 2
    pair_bank = [0] * npairs
    for c in range(npairs):
        pair_bank[c] = c % 3 if c < npairs - 3 else (c - (npairs - 3)) % 2
    bank_last = {}
    bank_first = {}
    for c, bk in enumerate(pair_bank):
        bank_last[bk] = c
        bank_first.setdefault(bk, c)
    return dict(sizes=sizes, paths=paths, p_tiles=p_tiles, npt=len(p_tiles),
                pair_bank=pair_bank, bank_last=bank_last,
                bank_first=bank_first)


def _build_program(tiles):
    key = ("v27", tiles)
    if key in _prog_cache:
        return _prog_cache[key]

    plan = _make_plan(tiles)
    sizes, paths = plan["sizes"], plan["paths"]
    npt = plan["npt"]

    f32 = mybir.dt.float32
    f16 = mybir.dt.float16
    f8 = mybir.dt.float8e4
    nc = bacc.Bacc(
        "TRN2",
        target_bir_lowering=False,
        debug=False,
        enable_asserts=False,
        num_devices=NCORES,
        enable_partition_id=False,
        monotonic_sem_count=0,
    )
    # one DRAM tensor of 512B-per-partition units: each block's segment is
    # [its hh tiles | its transposed packs], so ONE DMA per block delivers
    # both, alternating between the Sync and Scalar HWDGE queues
    units = tiles + npt
    blk = nc.dram_tensor("blk", [P, units, 2 * H], f8,
                         kind="ExternalInput").ap()
    # u broadcast row (512) | u chunk-major (4) | ones (1) | pad
    ubx = nc.dram_tensor("ubx", [P, 2 * H + 8], f8, kind="ExternalInput").ap()
    t_out = nc.dram_tensor("t_out", [1, 6 * H + 1], f32,
                           kind="ExternalOutput").ap()

    npairs = tiles // 2
    pair_bank = plan["pair_bank"]
    bank_last = plan["bank_last"]
    bank_first = plan["bank_first"]

    with tile.TileContext(nc) as tc:
        with (
            tc.tile_pool(name="singles", bufs=1) as singles,
            tc.tile_pool(name="blocks", bufs=6) as blocks,
            tc.tile_pool(name="small", bufs=2) as small,
            tc.tile_pool(name="scratch", bufs=4) as scratch,
            tc.tile_pool(name="psum", bufs=1, space="PSUM") as psum,
            tc.tile_pool(name="psc", bufs=2, space="PSUM") as psc,
        ):
            ubx_sb = singles.tile([P, 2 * H + 8], f8)
            u_sb = ubx_sb[:, 0:2 * H]
            ones_sb = ubx_sb[:, 2 * H + 4:2 * H + 5]
            p_grid = singles.tile([P, tiles, 16], f8)
            nshift = singles.tile([P, 1], f32)
            nc.gpsimd.memset(nshift, -SHIFT)

            t_banks = [psum.tile([1, 2 * H], f32, tag=f"tall{i}",
                                 name=f"tall{i}") for i in range(3)]
            s_ps = psum.tile([1, tiles], f32, tag="sps")

            t_sb = small.tile([1, 6 * H + 1], f32, tag="tsb")
            banks_copied = set()

            def emit_ws(pt0, psz, pbuf):
                for g in range(0, psz - 1, 2):
                    c = (pt0 + g) // 2
                    bk = pair_bank[c]
                    nc.tensor.matmul(
                        t_banks[bk],
                        lhsT=p_grid[:, pt0 + g:pt0 + g + 2, 0:1],
                        rhs=pbuf[:, g:g + 2, :],
                        start=(c == bank_first[bk]),
                        stop=(c == bank_last[bk]),
                        perf_mode=mybir.MatmulPerfMode.DoubleRow,
                    )
                    if c == bank_last[bk]:
                        # bank closed: drain it now so the copy overlaps
                        # the remaining stream
                        nc.vector.tensor_copy(
                            t_sb[0:1, 2 * H * bk:2 * H * (bk + 1)],
                            t_banks[bk])
                        banks_copied.add(bk)

            pending = []
            nblk = len(sizes)
            kP = 0   # global PE-tile serial (pack order)
            t0 = 0
            for b, sz in enumerate(sizes):
                nV, nP = paths[b]
                if b == 0:
                    nc.sync.dma_start(out=ubx_sb, in_=ubx)
                bt = blocks.tile([P, 2 * BMAX, 2 * H], f8)
                eng = nc.sync if b % 2 == 0 else nc.scalar
                eng.dma_start(out=bt[:, 0:sz + nP, :],
                              in_=blk[:, t0 + kP:t0 + kP + sz + nP, :])

                # deferred weighted sums first: they are ready (exp done
                # blocks ago) so PE never idles waiting for this block's data
                defer = 1 if b <= 2 else 2
                while len(pending) > defer:
                    emit_ws(*pending.pop(0))

                # DVE path: fused multiply + row-reduce per tile
                cblk = scratch.tile([P, BMAX], f32, tag="cblk")
                for g in range(nV):
                    sc = scratch.tile([P, 2 * H], f16, tag="sttout")
                    nc.vector.scalar_tensor_tensor(
                        out=sc,
                        in0=bt[:, g, :],
                        scalar=1.0,
                        in1=u_sb,
                        op0=mybir.AluOpType.mult,
                        op1=mybir.AluOpType.mult,
                        accum_out=cblk[:, g:g + 1],
                    )
                # PE path: per tile, 4 FWL weight loads + 1-col matmuls
                if nP > 0:
                    cp = psc.tile([P, BMAX], f32, tag="cp")
                    for j in range(nP):
                        for ch in range(4):
                            nc.tensor.matmul(
                                cp[:, j:j + 1],
                                lhsT=bt[:, sz + j, ch * P:(ch + 1) * P],
                                rhs=ubx_sb[:, 2 * H + ch:2 * H + ch + 1],
                                start=(ch == 0),
                                stop=(ch == 3),
                            )
                # batched exps -> fp8 p-grid (16B stride)
                if nV > 0:
                    nc.scalar.activation(
                        out=p_grid[:, t0:t0 + nV, 0],
                        in_=cblk[:, 0:nV],
                        func=mybir.ActivationFunctionType.Exp,
                        bias=nshift,
                        scale=1.0,
                    )
                if nP > 0:
                    nc.scalar.activation(
                        out=p_grid[:, t0 + nV:t0 + sz, 0],
                        in_=cp[:, 0:nP],
                        func=mybir.ActivationFunctionType.Exp,
                        bias=nshift,
                        scale=1.0,
                    )
                kP += nP
                pending.append((t0, sz, bt))
                t0 += sz

            while pending:
                emit_ws(*pending.pop(0))

            # s = sum(p): partition-reduce via ones-matmul over the strided
            # p-grid, then a tiny free-dim reduce on the [1, tiles] PSUM row
            nc.tensor.matmul(s_ps, lhsT=ones_sb, rhs=p_grid[:, 0:tiles, 0],
                             start=True, stop=True)
            for bk in range(3):
                if bk not in banks_copied:
                    nc.vector.tensor_copy(
                        t_sb[0:1, 2 * H * bk:2 * H * (bk + 1)], t_banks[bk])
            nc.vector.reduce_sum(out=t_sb[0:1, 6 * H:], in_=s_ps,
                                 axis=mybir.AxisListType.X)
            nc.sync.dma_start(out=t_out, in_=t_sb)

    nc.compile()
    _prog_cache[key] = (nc, plan)
    return nc, plan


def _run_device(h_static, h_dynamic, u_cat, valid_idx):
    """Stream the compacted valid rows through the 8-core SPMD kernel.

    Returns (t [2H] float64 summed over cores, s float64).  Pad rows
    contribute exactly zero (their fp8 p rounds to 0), so no correction.
    """
    global LAST_RESULTS

    nv = len(valid_idx)
    q = (nv + NCORES - 1) // NCORES
    tiles = max(1, (q + P - 1) // P)
    if tiles % 2:
        tiles += 1          # even count: DoubleRow pairs tiles, no odd path
    npad = P * tiles
    nc, plan = _build_program(tiles)
    p_tiles = plan["p_tiles"]
    npt = plan["npt"]

    u8 = u_cat.astype(FP8)
    ubx = np.zeros((P, 2 * H + 8), FP8)
    ubx[:, 0:2 * H] = u8
    ubx[:, 2 * H:2 * H + 4] = u8.reshape(4, P).T
    ubx[:, 2 * H + 4] = FP8(1.0)

    sizes, paths = plan["sizes"], plan["paths"]
    units = tiles + npt
    in_maps = []
    for c in range(NCORES):
        rows = valid_idx[c * q:(c + 1) * q]
        nr = len(rows)
        h8 = np.zeros((npad, 2 * H), FP8)
        if nr:
            h8[:nr, 0:H] = h_static[rows].astype(FP8)
            h8[:nr, H:2 * H] = h_dynamic[rows].astype(FP8)
        hview = h8.reshape(P, tiles, 2 * H)
        blk = np.zeros((P, units, 2 * H), FP8)
        t0 = kp = 0
        for b, sz in enumerate(sizes):
            nV, nP = paths[b]
            seg = t0 + kp
            blk[:, seg:seg + sz, :] = hview[:, t0:t0 + sz, :]
            for j in range(nP):
                tr = hview[:, t0 + nV + j, :]      # [P, 2H] rows of tile
                for ch in range(4):
                    blk[:, seg + sz + j, ch * P:(ch + 1) * P] = \
                        tr[:, ch * P:(ch + 1) * P].T
            t0 += sz
            kp += nP
        in_maps.append({"blk": blk, "ubx": ubx})

    res = bass_utils.run_bass_kernel_spmd(
        nc, in_maps, core_ids=list(range(NCORES)), **TRACE_OPTS
    )
    LAST_RESULTS = res

    t = np.zeros(2 * H, np.float64)
    s = 0.0
    for c in range(NCORES):
        arr = res.results[c]["t_out"].astype(np.float64)[0]
        t += arr[0:2 * H] + arr[2 * H:4 * H] + arr[4 * H:6 * H]
        s += arr[6 * H]
    return t, s


def kernel(
    h_dynamic,
    h_static,
    W_static_kvl,
    W_dyn_kvl,
    W_q,
    W1,
    b1,
    W2,
    b2,
    valid_mask,
    current_node,
):
    h_dynamic = np.asarray(h_dynamic, np.float32)
    h_static = np.asarray(h_static, np.float32)
    W_static_kvl = np.asarray(W_static_kvl, np.float32)
    W_dyn_kvl = np.asarray(W_dyn_kvl, np.float32)
    W_q = np.asarray(W_q, np.float32)
    W1 = np.asarray(W1, np.float32)
    b1 = np.asarray(b1, np.float32)
    W2 = np.asarray(W2, np.float32)
    b2 = np.asarray(b2, np.float32)
    valid = np.asarray(valid_mask).astype(bool)
    cur = int(current_node)

    scale = 1.0 / math.sqrt(H)

    # ---- tiny host-side prologue (exact math on one row) ----
    h_cur = (h_static[cur].astype(np.float64) + h_dynamic[cur].astype(np.float64))
    q = h_cur @ W_q.astype(np.float64)  # [H]
    u_s = (W_static_kvl[:, 0:H].astype(np.float64) @ q) * scale
    u_d = (W_dyn_kvl[:, 0:H].astype(np.float64) @ q) * scale
    u_cat = np.concatenate([u_s, u_d]).astype(np.float32)  # [2H]

    valid_idx = np.flatnonzero(valid)

    W_vs = W_static_kvl[:, H:2 * H].astype(np.float64)
    W_vd = W_dyn_kvl[:, H:2 * H].astype(np.float64)

    if len(valid_idx) == 0:
        # all-masked edge case: reference softmax degenerates to uniform
        # over all N nodes; context is the mean of V. The logit cancels in
        # the final output anyway; run the device on a dummy row for timing.
        t, s = _run_device(h_static, h_dynamic, u_cat, np.array([0]))
        context = (h_static.mean(0).astype(np.float64) @ W_vs
                   + h_dynamic.mean(0).astype(np.float64) @ W_vd)
    else:
        t, s = _run_device(h_static, h_dynamic, u_cat, valid_idx)
        context = (t[:H] @ W_vs + t[H:] @ W_vd) / s  # [H]

    # ---- tiny host-side epilogue ----
    fuse = np.concatenate([h_cur, context])  # [2H]
    hidden = np.maximum(fuse @ W1.astype(np.float64) + b1.astype(np.float64), 0.0)
    logit = float(hidden @ W2.astype(np.float64)[:, 0] + float(b2[0]))

    logits_all = np.where(valid, np.float32(logit), NEG).astype(np.float32)

    LAST_INTERNALS.update(
        dict(u_cat=u_cat, t=t, s=s, context=context, logit=logit)
    )

    # exact replication of the reference's sampling (jax threefry, key(1))
    import contextlib

    import jax
    import jax.numpy as jnp

    try:
        ctx = jax.default_device(jax.devices("cpu")[0])
    except Exception:
        ctx = contextlib.nullcontext()
    with ctx:
        logits_j = jnp.asarray(logits_all)
        choice = jax.random.categorical(jax.random.key(1), logits_j)
        log_probs = jax.nn.log_softmax(logits_j)
        log_prob = log_probs[choice]
        choice_np = np.asarray(choice)
        log_prob_np = np.asarray(log_prob)

    return (choice_np, log_prob_np)


# revision 17
# speedup vs baseline: 1.1485x; 1.0138x over previous
"""Trainium2 Bass kernel for nn_AttentionDecoder (N=100000, H=256, 8 cores).

Math reduction (same as the fp16 baseline)
------------------------------------------
With W_ks = W_static_kvl[:, :H] etc., the reference collapses to one pass
over the only large tensors (h_static, h_dynamic):

    compat   = h_s @ u_s + h_d @ u_d        with u_* = (W_k* @ q)/sqrt(H)
    p_i      = exp(compat_i - SHIFT)        (valid nodes only)
    context  = ((t_s) @ W_vs + (t_d) @ W_vd) / s,  t = sum_i p_i [h_s|h_d]_i

The host compacts to the ~50% valid rows before sharding.  Pad rows are
zero; exp(0-SHIFT)=e^-8=3.35e-4 rounds to 0 in the fp8 p-grid, so pads
contribute exactly nothing to t or s (no host-side pad correction).

v27 (fp8): the node stream is float8e4 (1 B/elem), halving HBM traffic vs
fp16.  Per 128x512 tile, compat runs on one of two paths:
  * DVE: one fused scalar_tensor_tensor (mult + row-reduce) vs a broadcast
    fp8 u row.  fp8 costs the same DVE cycles as fp16 (no 2x mode either
    way), so the byte halving is free.
  * PE: the host also ships those tiles TRANSPOSED (col-major chunks); each
    chunk is an FWL weight load + a 1-column matmul against u-chunk,
    accumulating compat for 128 rows in PSUM.  Sustained cost is ~25ns per
    chunk, so PE absorbs most tiles; the extra bytes (tile shipped twice)
    still total well under the fp16 single-copy budget.
Each block gets two batched exps (SBUF cblk for DVE tiles, PSUM cp for PE
tiles) writing p = exp(compat-SHIFT) straight into a 16B-strided fp8 p-grid
(the stride satisfies DoubleRow's lhsT step%16 rule).  Weighted sums then
run as fp8 DoubleRow matmuls: one matmul per PAIR of tiles (lhsT = two
p-columns, rhs = two adjacent tiles), ~2x the fp16 rate, rotating 3 PSUM
row-groups; s = sum(p) via a ones-column matmul over the strided p-grid.
DMA: hh blocks stream on the Sync HWDGE queue, the transposed packs +
singles on the GpSimd queue (otherwise idle).  Host runs the tiny MLP head
and exact jax sampling, as before.
"""

import math

import numpy as np
import ml_dtypes

import concourse.bacc as bacc
import concourse.mybir as mybir
import concourse.tile as tile
from concourse import bass_utils

# ---- problem constants (hardcoded per harness contract) ----
H = 256
NCORES = 8
P = 128                     # SBUF partitions
BMAX = 8                    # max tiles per DMA block
SHIFT = 8.0
NEG = np.float32(-1e9)
FP8 = ml_dtypes.float8_e4m3

# test.py hooks
TRACE_OPTS: dict = {}
LAST_RESULTS = None
LAST_INTERNALS: dict = {}

_prog_cache: dict = {}


def _make_plan(tiles):
    """Static schedule for a per-core tile count.

    Blocks have even sizes (DoubleRow pairs tiles within a block); a final
    odd tile gets its own 1-tile block.  Within each block the first nv
    tiles take the DVE compat path, the rest the PE (transposed) path.
    Returns dict with sizes, per-block (nv, np), p_tiles (global indices of
    PE-path tiles, in pack order).
    """
    assert tiles % 2 == 0 or tiles == 1
    # ramp: small all-DVE first block (its exp unblocks the first weighted
    # sums while the transposed packs are still in flight), fat middle,
    # small PE-only tail blocks (chunks drain fast)
    sizes = []
    rem = tiles
    if rem > 4:
        sizes.append(4)
        rem -= 4
    while rem > 6:
        s = min(BMAX, rem - 6)
        sizes.append(s)
        rem -= s
    if rem > 2:
        sizes.append(rem - 2)
        rem = 2
    if rem:
        sizes.append(rem)

    nblk = len(sizes)
    # global DVE-path share (engine balance; see module docstring)
    n_dve = int(round(0.40 * tiles))
    if tiles < 6:
        n_dve = tiles

    nv = [0] * nblk
    nv[0] = min(sizes[0], n_dve)
    k = nv[0]
    while k < n_dve:
        done = True
        for b in range(1, nblk - 1):
            if k >= n_dve:
                break
            if nv[b] < sizes[b] - 1:
                nv[b] += 1
                k += 1
                done = False
        if done:
            break
    n_dve = k

    paths = []
    p_tiles = []
    t0 = 0
    for b in range(nblk):
        npb = sizes[b] - nv[b]
        paths.append((nv[b], npb))
        for j in range(npb):
            p_tiles.append(t0 + nv[b] + j)
        t0 += sizes[b]

    # ws pair -> PSUM bank; bank 2 closes early so its output copy overlaps
    # the tail blocks
    npairs = tiles // 2
    pair_bank = [0] * npairs
    for c in range(npairs):
        pair_bank[c] = c % 3 if c < npairs - 3 else (c - (npairs - 3)) % 2
    bank_last = {}
    bank_first = {}
    for c, bk in enumerate(pair_bank):
        bank_last[bk] = c
        bank_first.setdefault(bk, c)
    return dict(sizes=sizes, paths=paths, p_tiles=p_tiles, npt=len(p_tiles),
                pair_bank=pair_bank, bank_last=bank_last,
                bank_first=bank_first)


def _build_program(tiles):
    key = ("v27", tiles)
    if key in _prog_cache:
        return _prog_cache[key]

    plan = _make_plan(tiles)
    sizes, paths = plan["sizes"], plan["paths"]
    npt = plan["npt"]

    f32 = mybir.dt.float32
    f16 = mybir.dt.float16
    f8 = mybir.dt.float8e4
    nc = bacc.Bacc(
        "TRN2",
        target_bir_lowering=False,
        debug=False,
        enable_asserts=False,
        num_devices=NCORES,
        enable_partition_id=False,
        monotonic_sem_count=0,
    )
    # one DRAM tensor of 512B-per-partition units: each block's segment is
    # [its hh tiles | its transposed packs], so ONE DMA per block delivers
    # both, alternating between the Sync and Scalar HWDGE queues
    units = tiles + npt
    blk = nc.dram_tensor("blk", [P, units, 2 * H], f8,
                         kind="ExternalInput").ap()
    # u broadcast row (512) | u chunk-major (4) | ones (1) | pad
    ubx = nc.dram_tensor("ubx", [P, 2 * H + 8], f8, kind="ExternalInput").ap()
    t_out = nc.dram_tensor("t_out", [1, 6 * H + 1], f32,
                           kind="ExternalOutput").ap()

    npairs = tiles // 2
    pair_bank = plan["pair_bank"]
    bank_last = plan["bank_last"]
    bank_first = plan["bank_first"]

    with tile.TileContext(nc) as tc:
        with (
            tc.tile_pool(name="singles", bufs=1) as singles,
            tc.tile_pool(name="blocks", bufs=6) as blocks,
            tc.tile_pool(name="small", bufs=2) as small,
            tc.tile_pool(name="scratch", bufs=4) as scratch,
            tc.tile_pool(name="psum", bufs=1, space="PSUM") as psum,
            tc.tile_pool(name="psc", bufs=2, space="PSUM") as psc,
        ):
            ubx_sb = singles.tile([P, 2 * H + 8], f8)
            u_sb = ubx_sb[:, 0:2 * H]
            ones_sb = ubx_sb[:, 2 * H + 4:2 * H + 5]
            p_grid = singles.tile([P, tiles, 16], f8)
            nshift = singles.tile([P, 1], f32)
            nc.gpsimd.memset(nshift, -SHIFT)

            t_banks = [psum.tile([1, 2 * H], f32, tag=f"tall{i}",
                                 name=f"tall{i}") for i in range(3)]
            s_ps = psum.tile([1, tiles], f32, tag="sps")

            t_sb = small.tile([1, 6 * H + 1], f32, tag="tsb")
            banks_copied = set()

            def emit_ws(pt0, psz, pbuf):
                for g in range(0, psz - 1, 2):
                    c = (pt0 + g) // 2
                    bk = pair_bank[c]
                    nc.tensor.matmul(
                        t_banks[bk],
                        lhsT=p_grid[:, pt0 + g:pt0 + g + 2, 0:1],
                        rhs=pbuf[:, g:g + 2, :],
                        start=(c == bank_first[bk]),
                        stop=(c == bank_last[bk]),
                        perf_mode=mybir.MatmulPerfMode.DoubleRow,
                    )
                    if c == bank_last[bk]:
                        # bank closed: drain it now so the copy overlaps
                        # the remaining stream
                        nc.vector.tensor_copy(
                            t_sb[0:1, 2 * H * bk:2 * H * (bk + 1)],
                            t_banks[bk])
                        banks_copied.add(bk)

            pending = []
            nblk = len(sizes)
            kP = 0   # global PE-tile serial (pack order)
            t0 = 0
            for b, sz in enumerate(sizes):
                nV, nP = paths[b]
                if b == 0:
                    nc.sync.dma_start(out=ubx_sb, in_=ubx)
                bt = blocks.tile([P, 2 * BMAX, 2 * H], f8)
                eng = nc.sync if b % 2 == 0 else nc.scalar
                eng.dma_start(out=bt[:, 0:sz + nP, :],
                              in_=blk[:, t0 + kP:t0 + kP + sz + nP, :])

                # deferred weighted sums first: they are ready (exp done
                # blocks ago) so PE never idles waiting for this block's data
                defer = 1 if b <= 2 else 2
                while len(pending) > defer:
                    emit_ws(*pending.pop(0))

                # DVE path: fused multiply + row-reduce per tile
                cblk = scratch.tile([P, BMAX], f32, tag="cblk")
                for g in range(nV):
                    sc = scratch.tile([P, 2 * H], f16, tag="sttout")
                    nc.vector.scalar_tensor_tensor(
                        out=sc,
                        in0=bt[:, g, :],
                        scalar=1.0,
                        in1=u_sb,
                        op0=mybir.AluOpType.mult,
                        op1=mybir.AluOpType.mult,
                        accum_out=cblk[:, g:g + 1],
                    )
                # PE path: per tile, 4 FWL weight loads + 1-col matmuls
                if nP > 0:
                    cp = psc.tile([P, BMAX], f32, tag="cp")
                    for j in range(nP):
                        for ch in range(4):
                            nc.tensor.matmul(
                                cp[:, j:j + 1],
                                lhsT=bt[:, sz + j, ch * P:(ch + 1) * P],
                                rhs=ubx_sb[:, 2 * H + ch:2 * H + ch + 1],
                                start=(ch == 0),
                                stop=(ch == 3),
                            )
                # batched exps -> fp8 p-grid (16B stride)
                if nV > 0:
                    nc.scalar.activation(
                        out=p_grid[:, t0:t0 + nV, 0],
                        in_=cblk[:, 0:nV],
                        func=mybir.ActivationFunctionType.Exp,
                        bias=nshift,
                        scale=1.0,
                    )
                if nP > 0:
                    nc.scalar.activation(
                        out=p_grid[:, t0 + nV:t0 + sz, 0],
                        in_=cp[:, 0:nP],
                        func=mybir.ActivationFunctionType.Exp,
                        bias=nshift,
                        scale=1.0,
                    )
                kP += nP
                pending.append((t0, sz, bt))
                t0 += sz

            while pending:
                emit_ws(*pending.pop(0))

            # s = sum(p): partition-reduce via ones-matmul over the strided
            # p-grid, then a tiny free-dim reduce on the [1, tiles] PSUM row
            nc.tensor.matmul(s_ps, lhsT=ones_sb, rhs=p_grid[:, 0:tiles, 0],
                             start=True, stop=True)
            for bk in range(3):
                if bk not in banks_copied:
                    nc.vector.tensor_copy(
                        t_sb[0:1, 2 * H * bk:2 * H * (bk + 1)], t_banks[bk])
            nc.vector.reduce_sum(out=t_sb[0:1, 6 * H:], in_=s_ps,
                                 axis=mybir.AxisListType.X)
            nc.sync.dma_start(out=t_out, in_=t_sb)

    nc.compile()
    _prog_cache[key] = (nc, plan)
    return nc, plan


def _run_device(h_static, h_dynamic, u_cat, valid_idx):
    """Stream the compacted valid rows through the 8-core SPMD kernel.

    Returns (t [2H] float64 summed over cores, s float64).  Pad rows
    contribute exactly zero (their fp8 p rounds to 0), so no correction.
    """
    global LAST_RESULTS

    nv = len(valid_idx)
    q = (nv + NCORES - 1) // NCORES
    tiles = max(1, (q + P - 1) // P)
    if tiles % 2:
        tiles += 1          # even count: DoubleRow pairs tiles, no odd path
    npad = P * tiles
    nc, plan = _build_program(tiles)
    p_tiles = plan["p_tiles"]
    npt = plan["npt"]

    u8 = u_cat.astype(FP8)
    ubx = np.zeros((P, 2 * H + 8), FP8)
    ubx[:, 0:2 * H] = u8
    ubx[:, 2 * H:2 * H + 4] = u8.reshape(4, P).T
    ubx[:, 2 * H + 4] = FP8(1.0)

    sizes, paths = plan["sizes"], plan["paths"]
    units = tiles + npt
    in_maps = []
    for c in range(NCORES):
        rows = valid_idx[c * q:(c + 1) * q]
        nr = len(rows)
        h8 = np.zeros((npad, 2 * H), FP8)
        if nr:
            h8[:nr, 0:H] = h_static[rows].astype(FP8)
            h8[:nr, H:2 * H] = h_dynamic[rows].astype(FP8)
        hview = h8.reshape(P, tiles, 2 * H)
        blk = np.zeros((P, units, 2 * H), FP8)
        t0 = kp = 0
        for b, sz in enumerate(sizes):
            nV, nP = paths[b]
            seg = t0 + kp
            blk[:, seg:seg + sz, :] = hview[:, t0:t0 + sz, :]
            for j in range(nP):
                tr = hview[:, t0 + nV + j, :]      # [P, 2H] rows of tile
                for ch in range(4):
                    blk[:, seg + sz + j, ch * P:(ch + 1) * P] = \
                        tr[:, ch * P:(ch + 1) * P].T
            t0 += sz
            kp += nP
        in_maps.append({"blk": blk, "ubx": ubx})

    res = bass_utils.run_bass_kernel_spmd(
        nc, in_maps, core_ids=list(range(NCORES)), **TRACE_OPTS
    )
    LAST_RESULTS = res

    t = np.zeros(2 * H, np.float64)
    s = 0.0
    for c in range(NCORES):
        arr = res.results[c]["t_out"].astype(np.float64)[0]
        t += arr[0:2 * H] + arr[2 * H:4 * H] + arr[4 * H:6 * H]
        s += arr[6 * H]
    return t, s


def kernel(
    h_dynamic,
    h_static,
    W_static_kvl,
    W_dyn_kvl,
    W_q,
    W1,
    b1,
    W2,
    b2,
    valid_mask,
    current_node,
):
    h_dynamic = np.asarray(h_dynamic, np.float32)
    h_static = np.asarray(h_static, np.float32)
    W_static_kvl = np.asarray(W_static_kvl, np.float32)
    W_dyn_kvl = np.asarray(W_dyn_kvl, np.float32)
    W_q = np.asarray(W_q, np.float32)
    W1 = np.asarray(W1, np.float32)
    b1 = np.asarray(b1, np.float32)
    W2 = np.asarray(W2, np.float32)
    b2 = np.asarray(b2, np.float32)
    valid = np.asarray(valid_mask).astype(bool)
    cur = int(current_node)

    scale = 1.0 / math.sqrt(H)

    # ---- tiny host-side prologue (exact math on one row) ----
    h_cur = (h_static[cur].astype(np.float64) + h_dynamic[cur].astype(np.float64))
    q = h_cur @ W_q.astype(np.float64)  # [H]
    u_s = (W_static_kvl[:, 0:H].astype(np.float64) @ q) * scale
    u_d = (W_dyn_kvl[:, 0:H].astype(np.float64) @ q) * scale
    u_cat = np.concatenate([u_s, u_d]).astype(np.float32)  # [2H]

    valid_idx = np.flatnonzero(valid)

    W_vs = W_static_kvl[:, H:2 * H].astype(np.float64)
    W_vd = W_dyn_kvl[:, H:2 * H].astype(np.float64)

    if len(valid_idx) == 0:
        # all-masked edge case: reference softmax degenerates to uniform
        # over all N nodes; context is the mean of V. The logit cancels in
        # the final output anyway; run the device on a dummy row for timing.
        t, s = _run_device(h_static, h_dynamic, u_cat, np.array([0]))
        context = (h_static.mean(0).astype(np.float64) @ W_vs
                   + h_dynamic.mean(0).astype(np.float64) @ W_vd)
    else:
        t, s = _run_device(h_static, h_dynamic, u_cat, valid_idx)
        context = (t[:H] @ W_vs + t[H:] @ W_vd) / s  # [H]

    # ---- tiny host-side epilogue ----
    fuse = np.concatenate([h_cur, context])  # [2H]
    hidden = np.maximum(fuse @ W1.astype(np.float64) + b1.astype(np.float64), 0.0)
    logit = float(hidden @ W2.astype(np.float64)[:, 0] + float(b2[0]))

    logits_all = np.where(valid, np.float32(logit), NEG).astype(np.float32)

    LAST_INTERNALS.update(
        dict(u_cat=u_cat, t=t, s=s, context=context, logit=logit)
    )

    # exact replication of the reference's sampling (jax threefry, key(1))
    import contextlib

    import jax
    import jax.numpy as jnp

    try:
        ctx = jax.default_device(jax.devices("cpu")[0])
    except Exception:
        ctx = contextlib.nullcontext()
    with ctx:
        logits_j = jnp.asarray(logits_all)
        choice = jax.random.categorical(jax.random.key(1), logits_j)
        log_probs = jax.nn.log_softmax(logits_j)
        log_prob = log_probs[choice]
        choice_np = np.asarray(choice)
        log_prob_np = np.asarray(log_prob)

    return (choice_np, log_prob_np)
